# revision 2
# baseline (speedup 1.0000x reference)
"""Trainium2 Bass kernel v2 for one pre-LN transformer block (B=4, T=2048,
C=1024, H=16, HS=64, FFN=4096, causal).

Sharding: 8 cores = (batch b) x (parity s). Core (b, s) owns query blocks
{2j+s}. HOST PERMUTES tokens own-first: xP = [own 1024 | other 1024], so
the program is parity-independent (only input DATA differs per core).

Key structure vs v1:
  - LN1 folded into QKV: the GEMMs run directly on fp8-quantized raw x;
    mean correction is a K=1 matmul appended to each chain (stationary =
    colsum(W) row), 1/std is fused into the PSUM->SBUF copies (DVE row
    mult for Q/K, per-token [P,1] tensor_scalar for V). Stats are scaled
    by SX internally (x8 = x*SX) so all fold constants stay f16-normal.
  - QKV + proj in fp8e4m3 DoubleRow (two K-chunks interleaved per matmul);
    attention and FFN stay fp16 (fp8 FFN fails the error budget).
  - ACT only does exp (one instr covers both heads of a pair) and relu;
    copies are on DVE/Pool; causal masking is a DVE 0/1 multiply on the
    diagonal 128-chunk (no PE mask matmuls).
  - Permuted order makes segment starts exact-causal. Rowsum via ones
    column in V; reciprocal batched over all 16 heads; LN2 rsqrt applied
    on the FFN2 output (commutes through relu and the W2 contraction).
"""

import sys

for _p in ("/opt/trn_rl_repo", "/root/.axon_site/_ro/trn_rl_repo"):
    if _p not in sys.path:
        sys.path.append(_p)

import json
from contextlib import ExitStack

import numpy as np
import ml_dtypes

import concourse.bass as bass
import concourse.tile as tile
from concourse import mybir
from concourse.bass_utils import run_bass_kernel_spmd
from concourse.masks import make_identity

F32 = mybir.dt.float32
F16 = mybir.dt.float16
F8 = mybir.dt.float8e4
AF = mybir.ActivationFunctionType
OP = mybir.AluOpType
DR = mybir.MatmulPerfMode.DoubleRow

B, T, C, H, HS = 4, 2048, 1024, 16, 64
P = 128
CB = C // P            # 8 feature blocks
KT = CB // 2           # 4 interleaved K-pair chunks for fp8 DR
TB = T // P            # 16 token blocks
TOWN = T // 2          # own tokens per core
FF = 4 * C             # 4096
FB = FF // P           # 32
LN_EPS = 1e-5

# fp8 scales (powers of two; host asserts amax fits)
SX = 32.0              # x activations
SW = 1024.0            # all fp8 weights
SXO = 32.0             # attention output oT
ALPHA = 16.0           # extra kT/qT/v scale keeping f16 rows out of subnormals
CW = ALPHA / SW        # folds into r-scale rows (r comes out as r/SX)
EXP_SCALE = 0.125 / (ALPHA * ALPHA)
CP = 1.0 / (SXO * SW)  # proj dequant

_patched = False


def _install_wait_split():
    """Toolchain workarounds (same as v1): split multi-waits; drop the
    advisory birverifier pass."""
    global _patched
    if _patched:
        return
    _patched = True
    orig = bass.Bass.to_json_bytes

    def patched(self, *a, **kw):
        doc = json.loads(orig(self, *a, **kw))
        changed = False
        for f in doc.get("functions", []):
            for bb in f.get("basic_blocks", f.get("blocks", [])):
                out = []
                for inst in bb.get("instructions", []):
                    si = inst.get("sync_info")
                    waits = (si or {}).get("on_wait", [])
                    if len(waits) > 1:
                        changed = True
                        for k, w in enumerate(waits[:-1]):
                            out.append(
                                {
                                    "debug": inst.get("debug", 0),
                                    "engine": inst["engine"],
                                    "ins": [],
                                    "name": f"{inst['name']}_w{k}",
                                    "opcode": "EventSemaphore",
                                    "outs": [],
                                    "sync_info": {"on_update": [], "on_wait": [w]},
                                }
                            )
                        si["on_wait"] = waits[-1:]
                    out.append(inst)
                bb["instructions"] = out
        return json.dumps(doc).encode() if changed else orig(self, *a, **kw)

    bass.Bass.to_json_bytes = patched

    import concourse.bass_utils as bu

    orig_run = bu.run_command

    def patched_run(argv, **kw):
        argv = list(argv)
        for i, a in enumerate(argv):
            if isinstance(a, str) and a.startswith("birverifier,"):
                argv[i] = a[len("birverifier,"):]
        return orig_run(argv, **kw)

    bu.run_command = patched_run


def _seg512(lo, hi):
    segs = []
    while lo < hi:
        nxt = min(hi, (lo // 512 + 1) * 512)
        segs.append((lo, nxt))
        lo = nxt
    return segs


def build_nc(repeat=1, stop_after=None):
    nc = bass.Bass(target_bir_lowering=False)

    x8 = nc.dram_tensor("x8", [P, KT, 2, T], F8, kind="ExternalInput")
    xo16 = nc.dram_tensor("xo16", [C, TOWN], F16, kind="ExternalInput")
    wq8 = nc.dram_tensor("wq8", [P, KT, 2, C], F8, kind="ExternalInput")
    wk8 = nc.dram_tensor("wk8", [P, KT, 2, C], F8, kind="ExternalInput")
    wv8 = nc.dram_tensor("wv8", [P, KT, 2, C], F8, kind="ExternalInput")
    wp8 = nc.dram_tensor("wp8", [P, KT, 2, C], F8, kind="ExternalInput")
    wqs = nc.dram_tensor("wqs", [1, C], F16, kind="ExternalInput")
    wks = nc.dram_tensor("wks", [1, C], F16, kind="ExternalInput")
    wvs = nc.dram_tensor("wvs", [1, C], F16, kind="ExternalInput")
    w1 = nc.dram_tensor("w1", [FB, C, P], F16, kind="ExternalInput")
    w1s = nc.dram_tensor("w1s", [1, FF], F16, kind="ExternalInput")
    w2 = nc.dram_tensor("w2", [FF, C], F16, kind="ExternalInput")
    msk = nc.dram_tensor("msk", [2, P, 2, P], F16, kind="ExternalInput")
    outT = nc.dram_tensor("outT", [C, TOWN], F32, kind="ExternalOutput")

    with tile.TileContext(nc) as tc, ExitStack() as _rep, ExitStack() as top:
        if repeat > 1:
            _rep.enter_context(tc.For_i(0, repeat, 1))
        const = top.enter_context(tc.tile_pool(name="const", bufs=1, side="left"))
        ident16 = const.tile([P, P], F16, tag="ident")
        make_identity(nc, ident16)
        tri2 = const.tile([P, 2, P], F16, tag="tri2")
        pm2 = const.tile([P, 2, P], F16, tag="pm2")
        nc.sync.dma_start(out=tri2, in_=msk[0])
        nc.sync.dma_start(out=pm2, in_=msk[1])
        negon = const.tile([P, 1], F16, tag="negon")   # -1/C (stats on x8)
        poson = const.tile([P, 1], F16, tag="poson")   # sq tiles hold (x8/8)^2
        nc.vector.memset(negon, -1.0 / C)
        nc.vector.memset(poson, 64.0 / C)
        poson2 = const.tile([P, 1], F16, tag="poson2")
        nc.vector.memset(poson2, 1.0 / C)
        crow = const.tile([1, P], F16, tag="crow")     # ALPHA/SW for Q/K r-rows
        one_row = const.tile([1, P], F16, tag="onerow")
        nc.vector.memset(crow, CW)
        nc.vector.memset(one_row, 1.0)
        one_f32 = const.tile([1, 1], F32, tag="onef32")
        nc.vector.memset(one_f32, 1.0)
        eps1_sb = const.tile([1, 1], F32, tag="eps1")  # stats carry SX^2
        eps2_sb = const.tile([1, 1], F32, tag="eps2")
        nc.vector.memset(eps1_sb, LN_EPS * SX * SX)
        nc.vector.memset(eps2_sb, LN_EPS)

        def ln_rows(pool, nt, label):
            """Stats rows as [1, nt] single-partition tiles."""
            mneg = pool.tile([1, nt], F32, tag=f"m_{label}")
            work = pool.tile([1, nt], F32, tag=f"w_{label}")
            m16 = pool.tile([1, nt], F16, tag=f"m16_{label}")
            r16 = pool.tile([1, nt], F16, tag=f"r16_{label}")
            return mneg, work, m16, r16

        def ln_var_chain(mneg, work, m16, r16, eps_sb):
            # in-place: m16=mneg; mneg<-mneg^2; work<-sqrt(work-mneg^2+eps);
            # work<-1/work (= rinv); r16=work
            nc.vector.tensor_copy(m16[:], mneg[:])
            nc.vector.tensor_tensor(out=mneg[:], in0=mneg[:], in1=mneg[:], op=OP.mult)
            nc.vector.tensor_tensor(out=work[:], in0=work[:], in1=mneg[:], op=OP.subtract)
            nc.scalar.activation(work[:], work[:], AF.Sqrt, bias=eps_sb[0:1, 0:1])
            nc.vector.reciprocal(out=work[:], in_=work[:])
            nc.vector.tensor_copy(r16[:], work[:])

        def replicate(row_ap, out_tile, out_sl, stat_row, parts, ps_pool, name):
            """out_tile[0:parts, out_sl] = stat_row_value * row_ap via K=1 PE."""
            rp = ps_pool.tile([parts, 512], F32, tag="repps", name=f"rp_{name}")
            nc.tensor.matmul(rp[0:parts, :], stat_row[0:1, 0:parts], row_ap,
                             start=True, stop=True)
            nc.scalar.activation(out_tile[0:parts, out_sl], rp[0:parts, :], AF.Copy)

        # Right-side pool stack, entered in REVERSE close order:
        # proj_grp & att_grp (close after proj) < qkv_grp (close after
        # attention) < x8_grp (close after QKV).
        proj_grp = ExitStack()
        wp_pool = proj_grp.enter_context(tc.tile_pool(name="wp8", bufs=1, side="right"))
        xo_pool = proj_grp.enter_context(tc.tile_pool(name="xo", bufs=CB, side="right"))
        att_grp = ExitStack()
        oT8_pool = att_grp.enter_context(tc.tile_pool(name="oT8", bufs=1, side="right"))
        qkv_grp = ExitStack()
        qT_pool = qkv_grp.enter_context(tc.tile_pool(name="qT", bufs=CB, side="right"))
        kT_pool = qkv_grp.enter_context(tc.tile_pool(name="kT", bufs=CB, side="right"))
        v_pool = qkv_grp.enter_context(tc.tile_pool(name="v", bufs=TB, side="right"))
        x8_grp = ExitStack()
        x8sb_pool = x8_grp.enter_context(tc.tile_pool(name="x8sb", bufs=1, side="right"))
        x8sb = x8sb_pool.tile([P, KT, 2, T], F8, tag="x8")
        nc.sync.dma_start(out=x8sb, in_=x8[:, :, :, :])

        # raw stats on x8 = SX*x: mneg1 = -SX*m, work1 = SX^2*E[x^2],
        # r1 = 1/(SX*std); all fold constants account for this.
        rows1 = x8_grp.enter_context(tc.tile_pool(name="rows1", bufs=1, side="right"))
        mneg1, work1, m1_16, r1_16 = ln_rows(rows1, T, "1")
        r1c = rows1.tile([P, TB], F32, tag="r1c")

        with tc.tile_pool(name="lnps", bufs=4, space="PSUM") as lnps, \
             tc.tile_pool(name="lnsq", bufs=2, side="right") as sqp:
            for n in range(4):
                sl = slice(n * 512, (n + 1) * 512)
                ps = lnps.tile([1, 512], F32, tag="st", name=f"l1a_{n}")
                ps2 = lnps.tile([1, 512], F32, tag="st", name=f"l1b_{n}")
                for t2 in range(KT):
                    for i in range(2):
                        nc.tensor.matmul(
                            ps[:], negon[:], x8sb[:, t2, i, sl],
                            start=(t2 == 0 and i == 0), stop=(t2 == KT - 1 and i == 1),
                        )
                        x8h = sqp.tile([P, 512], F8, tag="x8h", name=f"x8h_{n}_{t2}_{i}")
                        nc.scalar.activation(x8h[:], x8sb[:, t2, i, sl], AF.Copy,
                                             scale=0.125)
                        sq = sqp.tile([P, 512], F16, tag="sq", name=f"sq1_{n}_{t2}_{i}")
                        nc.vector.tensor_tensor(
                            out=sq[:], in0=x8h[:], in1=x8h[:], op=OP.mult,
                        )
                        nc.tensor.matmul(
                            ps2[:], poson[:], sq[:],
                            start=(t2 == 0 and i == 0), stop=(t2 == KT - 1 and i == 1),
                        )
                nc.scalar.activation(mneg1[0:1, sl], ps[:], AF.Copy)
                nc.scalar.activation(work1[0:1, sl], ps2[:], AF.Copy)
        ln_var_chain(mneg1, work1, m1_16, r1_16, eps1_sb)
        r1 = work1  # rinv lives in work1 after the chain

        # r1 as per-token columns for V (scaled by 1/SW): 16 PE transposes
        with tc.tile_pool(name="trps", bufs=1, space="PSUM") as trps:
            tcol = trps.tile([P, TB], F32, tag="tc")
            for tb in range(TB):
                nc.tensor.matmul(
                    tcol[:, tb : tb + 1], r1[0:1, tb * P : (tb + 1) * P],
                    one_f32[0:1, 0:1],
                    is_transpose=True, start=True, stop=True,
                )
            nc.vector.tensor_scalar(
                out=r1c[:], in0=tcol[:], scalar1=CW, scalar2=None, op0=OP.mult
            )

        # replicated r rows for Q (own segs 0-1) and K (full), dequant folded
        reps = top.enter_context(tc.tile_pool(name="reps", bufs=1, side="left"))
        Rf = reps.tile([P, T], F16, tag="Rf")
        with tc.tile_pool(name="repps", bufs=2, space="PSUM") as repps:
            for n in range(4):
                sl = slice(n * 512, (n + 1) * 512)
                replicate(r1_16[0:1, sl], Rf, sl, crow, P, repps, f"Rf{n}")

        # ---- QKV GEMMs (fp8 DoubleRow chains + K=1 mean fold) ----
        wrow_pool = top.enter_context(tc.tile_pool(name="wrows", bufs=1, side="left"))
        wqs_sb = wrow_pool.tile([1, C], F16, tag="wqs")
        wks_sb = wrow_pool.tile([1, C], F16, tag="wks")
        wvs_sb = wrow_pool.tile([1, C], F16, tag="wvs")
        nc.sync.dma_start(out=wqs_sb, in_=wqs[:])
        nc.sync.dma_start(out=wks_sb, in_=wks[:])
        nc.sync.dma_start(out=wvs_sb, in_=wvs[:])
        # (w1s loads later, into rows2)

        qT_t = [qT_pool.tile([P, TOWN], F16, tag="qT", name=f"qT_{i}") for i in range(CB)]
        kT_t = [kT_pool.tile([P, T], F16, tag="kT", name=f"kT_{i}") for i in range(CB)]
        v_t = [v_pool.tile([P, 2, 8, HS + 1], F16, tag="v", name=f"v_{i}") for i in range(TB)]

        with ExitStack() as wgrp:
            w8_pool = wgrp.enter_context(tc.tile_pool(name="w8", bufs=2, side="right"))
            wk_sb = w8_pool.tile([P, KT, 2, C], F8, tag="w8", name="wk8sb")
            wq_sb = w8_pool.tile([P, KT, 2, C], F8, tag="w8", name="wq8sb")
            nc.sync.dma_start(out=wk_sb, in_=wk8[:, :, :, :])
            nc.sync.dma_start(out=wq_sb, in_=wq8[:, :, :, :])

            with tc.tile_pool(name="qkvps", bufs=6, space="PSUM") as qkvps:
                # K over full batch
                for p in range(CB):
                    po = slice(p * P, (p + 1) * P)
                    for n in range(4):
                        sl = slice(n * 512, (n + 1) * 512)
                        ps = qkvps.tile([P, 512], F32, tag="ps", name=f"kps_{p}_{n}")
                        for t2 in range(KT):
                            nc.tensor.matmul(
                                ps[:], wk_sb[:, t2, :, po], x8sb[:, t2, :, sl],
                                start=(t2 == 0), stop=False, perf_mode=DR,
                            )
                        nc.tensor.matmul(
                            ps[:], wks_sb[0:1, po], m1_16[0:1, sl],
                            start=False, stop=True,
                        )
                        nc.vector.tensor_tensor(
                            out=kT_t[p][:, sl], in0=ps[:], in1=Rf[:, sl], op=OP.mult
                        )
                # wv reuses wk's buffer once K is done
                wv_sb = w8_pool.tile([P, KT, 2, C], F8, tag="w8", name="wv8sb")
                nc.sync.dma_start(out=wv_sb, in_=wv8[:, :, :, :])
                # Q over own tokens (first 2 segs in permuted order)
                for p in range(CB):
                    po = slice(p * P, (p + 1) * P)
                    for n in range(2):
                        sl = slice(n * 512, (n + 1) * 512)
                        ps = qkvps.tile([P, 512], F32, tag="ps", name=f"qps_{p}_{n}")
                        for t2 in range(KT):
                            nc.tensor.matmul(
                                ps[:], wq_sb[:, t2, :, po], x8sb[:, t2, :, sl],
                                start=(t2 == 0), stop=False, perf_mode=DR,
                            )
                        nc.tensor.matmul(
                            ps[:], wqs_sb[0:1, po], m1_16[0:1, sl],
                            start=False, stop=True,
                        )
                        nc.vector.tensor_tensor(
                            out=qT_t[p][:, sl], in0=ps[:], in1=Rf[:, sl], op=OP.mult
                        )
                # V: stationary = x8 token chunk, moving = wv8 (8 heads/matmul)
                for tb in range(TB):
                    tsl = slice(tb * P, (tb + 1) * P)
                    mrow = m1_16[0:1, tb * P : (tb + 1) * P]
                    for hh in range(2):
                        hsl = slice(hh * 512, (hh + 1) * 512)
                        ps = qkvps.tile([P, 512], F32, tag="ps", name=f"vps_{tb}_{hh}")
                        for t2 in range(KT):
                            nc.tensor.matmul(
                                ps[:], x8sb[:, t2, :, tsl], wv_sb[:, t2, :, hsl],
                                start=(t2 == 0), stop=False, perf_mode=DR,
                            )
                        nc.tensor.matmul(
                            ps[:], mrow, wvs_sb[0:1, hsl],
                            start=False, stop=True,
                        )
                        nc.vector.tensor_scalar(
                            out=v_t[tb][:, hh, :, 0:HS],
                            in0=ps[:].rearrange("p (h d) -> p h d", h=8),
                            scalar1=r1c[:, tb : tb + 1], scalar2=None, op0=OP.mult,
                        )
                    nc.vector.memset(v_t[tb][:, :, :, HS : HS + 1], ALPHA)
        x8_grp.close()  # x8 SBUF freed after QKV

        if stop_after == "qkv":
            with tc.tile_pool(name="dbg", bufs=2, side="right") as dbg:
                for p in range(CB):
                    ob = dbg.tile([P, TOWN], F32, tag="o", name=f"dbgq_{p}")
                    nc.vector.tensor_copy(ob[:], kT_t[p][:, 0:TOWN])
                    nc.sync.dma_start(out=outT[p * P : (p + 1) * P, :], in_=ob[:])
            qkv_grp.close(); att_grp.close(); proj_grp.close()
            return nc

        # prefetch proj weights + residual x during attention
        wp_sb = wp_pool.tile([P, KT, 2, C], F8, tag="wp8")
        nc.sync.dma_start(out=wp_sb, in_=wp8[:, :, :, :])
        xo_t = [xo_pool.tile([P, TOWN], F16, tag="xo", name=f"xo_{c}") for c in range(CB)]
        for c in range(CB):
            nc.sync.dma_start(out=xo_t[c], in_=xo16[c * P : (c + 1) * P, :])

        # =========================== attention ============================
        oT8 = oT8_pool.tile([P, CB, TOWN], F8, tag="oT8")

        with tc.tile_pool(name="stps", bufs=2, space="PSUM") as stps, \
             tc.tile_pool(name="otps", bufs=2, space="PSUM") as otps, \
             tc.tile_pool(name="pt", bufs=4, side="right") as pt_pool, \
             tc.tile_pool(name="attnsb", bufs=3, side="right") as attnsb:
            for p in range(CB):
                ot = [
                    otps.tile([HS + 1, TOWN], F32, tag="ot", name=f"ot_{p}_{g}")
                    for g in range(2)
                ]
                for i in range(TB):
                    q0 = i * P if i < 8 else (i - 8) * P
                    first = True
                    for (lo, hi) in _seg512(q0, TOWN):
                        w = hi - lo
                        st = stps.tile([P, 2, 512], F32, tag="st", name=f"st_{p}_{i}_{lo}")
                        for g in range(2):
                            off = g * 64
                            nc.tensor.matmul(
                                st[:, g, 0:w],
                                kT_t[p][off : off + 64, i * P : (i + 1) * P],
                                qT_t[p][off : off + 64, lo:hi],
                                start=True, stop=True, skip_group_check=True,
                            )
                        pt = pt_pool.tile([P, 2, 512], F16, tag="pt", name=f"pt_{p}_{i}_{lo}")
                        nc.scalar.activation(
                            pt[:, :, 0:w], st[:, :, 0:w], AF.Exp, scale=EXP_SCALE
                        )
                        if first:
                            m = tri2 if i < 8 else pm2
                            nc.vector.tensor_tensor(
                                out=pt[:, :, 0:P], in0=pt[:, :, 0:P], in1=m[:],
                                op=OP.mult,
                            )
                            first = False
                        for g in range(2):
                            h = 2 * p + g
                            nc.tensor.matmul(
                                ot[g][:, lo:hi], v_t[i][:, h // 8, h % 8, :],
                                pt[:, g, 0:w],
                                start=(i == 0), stop=(i == TB - 1),
                                skip_group_check=True,
                            )
                # per-pair normalize: rec = SXO/rowsum, oT8 = oraw*rep(rec)
                for g in range(2):
                    h = 2 * p + g
                    off = g * 64
                    recf = attnsb.tile([1, TOWN], F32, tag="rec", name=f"rec_{h}")
                    nc.vector.reciprocal(out=recf[:], in_=ot[g][64:65, :])
                    rec16g = attnsb.tile([1, TOWN], F16, tag="rec16", name=f"rec16_{h}")
                    nc.vector.tensor_scalar(
                        out=rec16g[:], in0=recf[:], scalar1=SXO, scalar2=None,
                        op0=OP.mult,
                    )
                    rep = attnsb.tile([64, TOWN], F16, tag="rep", name=f"rep_{h}")
                    for n in range(2):
                        sl = slice(n * 512, (n + 1) * 512)
                        rp = stps.tile([P, 2, 512], F32, tag="st", name=f"rp_{h}_{n}")
                        nc.tensor.matmul(
                            rp[0:64, 0, :], one_row[0:1, 0:64], rec16g[0:1, sl],
                            start=True, stop=True,
                        )
                        nc.scalar.activation(rep[:, sl], rp[0:64, 0, :], AF.Copy)
                    nc.vector.tensor_tensor(
                        out=oT8[off : off + 64, p, :],
                        in0=ot[g][0:64, :], in1=rep[:], op=OP.mult,
                    )

        qkv_grp.close()  # qT/kT/v freed

        if stop_after == "attn":
            with tc.tile_pool(name="dbg2", bufs=2, side="right") as dbg:
                for p in range(CB):
                    ob = dbg.tile([P, TOWN], F32, tag="o", name=f"dbga_{p}")
                    nc.vector.tensor_copy(ob[:], oT8[:, p, :])
                    nc.sync.dma_start(out=outT[p * P : (p + 1) * P, :], in_=ob[:])
            att_grp.close()
            proj_grp.close()
            return nc

        # ======================== proj + residual =========================
        res1_pool = top.enter_context(tc.tile_pool(name="res1", bufs=CB, side="left"))
        res1_t = [res1_pool.tile([P, TOWN], F16, tag="res1", name=f"res1_{i}") for i in range(CB)]
        with tc.tile_pool(name="saps", bufs=6, space="PSUM") as saps, \
             tc.tile_pool(name="sa16p", bufs=4, side="right") as sa16p:
            for cp in range(CB):
                po = slice(cp * P, (cp + 1) * P)
                for n in range(2):
                    sl = slice(n * 512, (n + 1) * 512)
                    ps = saps.tile([P, 512], F32, tag="sa", name=f"saps_{cp}_{n}")
                    for t2 in range(KT):
                        nc.tensor.matmul(
                            ps[:], wp_sb[:, t2, :, po],
                            oT8[:, 2 * t2 : 2 * t2 + 2, sl],
                            start=(t2 == 0), stop=(t2 == KT - 1), perf_mode=DR,
                        )
                    sa16 = sa16p.tile([P, 512], F16, tag="s16", name=f"sa16_{cp}_{n}")
                    nc.scalar.activation(sa16[:], ps[:], AF.Copy, scale=CP)
                    nc.vector.tensor_tensor(
                        out=res1_t[cp][:, sl], in0=sa16[:], in1=xo_t[cp][:, sl],
                        op=OP.add,
                    )
        att_grp.close()
        proj_grp.close()

        if stop_after == "proj":
            with tc.tile_pool(name="dbg3", bufs=2, side="right") as dbg:
                for c in range(CB):
                    ob = dbg.tile([P, TOWN], F32, tag="o", name=f"dbgp_{c}")
                    nc.vector.tensor_copy(ob[:], res1_t[c][:])
                    nc.sync.dma_start(out=outT[c * P : (c + 1) * P, :], in_=ob[:])
            return nc

        # ========================= LN2 + FFN1 =============================
        rows2 = top.enter_context(tc.tile_pool(name="rows2", bufs=1, side="left"))
        mneg2, work2, m2_16, r2_16 = ln_rows(rows2, TOWN, "2")
        with tc.tile_pool(name="ln2ps", bufs=4, space="PSUM") as lnps2, \
             tc.tile_pool(name="ln2sq", bufs=4, side="right") as sqp2:
            for n in range(2):
                sl = slice(n * 512, (n + 1) * 512)
                ps = lnps2.tile([1, 512], F32, tag="st", name=f"l2a_{n}")
                ps2 = lnps2.tile([1, 512], F32, tag="st", name=f"l2b_{n}")
                for c in range(CB):
                    nc.tensor.matmul(
                        ps[:], negon[:], res1_t[c][:, sl],
                        start=(c == 0), stop=(c == CB - 1),
                    )
                    sq = sqp2.tile([P, 512], F16, tag="sq", name=f"sq2_{n}_{c}")
                    nc.vector.tensor_tensor(
                        out=sq[:], in0=res1_t[c][:, sl], in1=res1_t[c][:, sl],
                        op=OP.mult,
                    )
                    nc.tensor.matmul(
                        ps2[:], poson2[:], sq[:],
                        start=(c == 0), stop=(c == CB - 1),
                    )
                nc.scalar.activation(mneg2[0:1, sl], ps[:], AF.Copy)
                nc.scalar.activation(work2[0:1, sl], ps2[:], AF.Copy)
        ln_var_chain(mneg2, work2, m2_16, r2_16, eps2_sb)

        R2 = reps.tile([P, TOWN], F16, tag="R2")
        with tc.tile_pool(name="r2ps", bufs=2, space="PSUM") as r2ps:
            for n in range(2):
                sl = slice(n * 512, (n + 1) * 512)
                replicate(r2_16[0:1, sl], R2, sl, one_row, P, r2ps, f"R2{n}")

        w1s_sb = rows2.tile([1, FF], F16, tag="w1s")
        nc.sync.dma_start(out=w1s_sb, in_=w1s[:])

        relu_pool = top.enter_context(tc.tile_pool(name="relu", bufs=FB, side="left"))
        relu_t = [relu_pool.tile([P, TOWN], F16, tag="relu", name=f"relu_{i}") for i in range(FB)]
        w2_pool = top.enter_context(tc.tile_pool(name="w2sb", bufs=4, side="right"))
        with tc.tile_pool(name="w1fp", bufs=3, side="right") as w1f_pool, \
             tc.tile_pool(name="ups", bufs=6, space="PSUM") as ups:
            for fc in range(FB):
                w1f = w1f_pool.tile([P, CB, P], F16, tag="w1f", name=f"w1f_{fc}")
                nc.sync.dma_start(
                    out=w1f, in_=w1[fc].rearrange("(cb p) f -> p cb f", p=P)
                )
                for n in range(2):
                    sl = slice(n * 512, (n + 1) * 512)
                    ps = ups.tile([P, 512], F32, tag="u", name=f"ups_{fc}_{n}")
                    for c in range(CB):
                        nc.tensor.matmul(
                            ps[:], w1f[:, c, :], res1_t[c][:, sl],
                            start=(c == 0), stop=False,
                        )
                    nc.tensor.matmul(
                        ps[:], w1s_sb[0:1, fc * P : (fc + 1) * P],
                        m2_16[0:1, sl],
                        start=False, stop=True,
                    )
                    nc.scalar.activation(relu_t[fc][:, sl], ps[:], AF.Relu)

        if stop_after == "ffn1":
            with tc.tile_pool(name="dbg4", bufs=2, side="right") as dbg:
                for c in range(CB):
                    ob = dbg.tile([P, TOWN], F32, tag="o", name=f"dbgf_{c}")
                    nc.vector.tensor_copy(ob[:], relu_t[c][:])
                    nc.sync.dma_start(out=outT[c * P : (c + 1) * P, :], in_=ob[:])
            return nc

        # ============================ FFN2 ================================
        with tc.tile_pool(name="ffps", bufs=1, space="PSUM") as ffps, \
             tc.tile_pool(name="osb", bufs=4, side="right") as osb_pool:
            for th in range(2):
                sl = slice(th * 512, (th + 1) * 512)
                pss = [
                    ffps.tile([P, 512], F32, tag=f"ff{cp}", name=f"ffps_{th}_{cp}")
                    for cp in range(CB)
                ]
                for fc in range(FB):
                    w2t = w2_pool.tile([P, C], F16, tag="w2", name=f"w2_{th}_{fc}")
                    nc.sync.dma_start(out=w2t, in_=w2[fc * P : (fc + 1) * P, :])
                    for cp in range(CB):
                        nc.tensor.matmul(
                            pss[cp][:], w2t[:, cp * P : (cp + 1) * P],
                            relu_t[fc][:, sl],
                            start=(fc == 0), stop=(fc == FB - 1),
                        )
                for cp in range(CB):
                    ff16 = osb_pool.tile([P, 512], F16, tag="f16", name=f"ff16_{th}_{cp}")
                    nc.vector.tensor_tensor(
                        out=ff16[:], in0=pss[cp][:], in1=R2[:, sl], op=OP.mult
                    )
                    ob = osb_pool.tile([P, 512], F32, tag="ob", name=f"ob_{th}_{cp}")
                    nc.vector.tensor_tensor(
                        out=ob[:], in0=ff16[:], in1=res1_t[cp][:, sl], op=OP.add
                    )
                    nc.sync.dma_start(out=outT[cp * P : (cp + 1) * P, sl], in_=ob[:])

    return nc


# ---------------------------------------------------------------------------
# host side
# ---------------------------------------------------------------------------

E4NP = ml_dtypes.float8_e4m3  # trn2 fp8e4 is the IEEE-ish variant (max 240)


def _q8(a, scale):
    assert np.abs(a).max() * scale < 224.0, (float(np.abs(a).max()), scale)
    return (a * scale).astype(E4NP)


def _il8(m, scale):
    """[C, N] -> fp8 interleaved [P, KT, 2, N]."""
    Cd, N = m.shape
    assert Cd == C
    r = m.reshape(KT, 2, P, N).transpose(2, 0, 1, 3)
    return np.ascontiguousarray(_q8(r, scale))


def _host_prep(inputs):
    x = np.asarray(inputs["x"], np.float32)
    Wq = np.asarray(inputs["Wq"], np.float32)
    Wk = np.asarray(inputs["Wk"], np.float32)
    Wv = np.asarray(inputs["Wv"], np.float32)
    Wp = np.asarray(inputs["Wproj"], np.float32)
    W1 = np.asarray(inputs["W1"], np.float32)
    W2 = np.asarray(inputs["W2"], np.float32)

    wq2 = Wq.transpose(1, 0, 2).reshape(C, C)
    wk2 = Wk.transpose(1, 0, 2).reshape(C, C)
    wv2 = Wv.transpose(1, 0, 2).reshape(C, C)

    wq8 = _il8(wq2, SW)
    wk8 = _il8(wk2, SW)
    wv8 = _il8(wv2, SW)
    wp8 = _il8(Wp, SW)
    # mean-fold rows: chains produce SW*SX*(W^T x); m1_16 = -SX*m, so these
    # rows are colsum(W)*SW to land at SW*SX*(-m*colsum).
    wqs_r = np.ascontiguousarray((wq2.sum(0, dtype=np.float64) * SW).astype(np.float16)[None])
    wks_r = np.ascontiguousarray((wk2.sum(0, dtype=np.float64) * SW).astype(np.float16)[None])
    wvs_r = np.ascontiguousarray((wv2.sum(0, dtype=np.float64) * SW).astype(np.float16)[None])

    w1r = np.ascontiguousarray(W1.reshape(C, FB, P).transpose(1, 0, 2).astype(np.float16))
    w1sum = np.ascontiguousarray(W1.sum(0, dtype=np.float64).astype(np.float16)[None])
    w2h = np.ascontiguousarray(W2.astype(np.float16))

    tri01 = (np.arange(P)[:, None] <= np.arange(P)[None, :]).astype(np.float16)
    tri2 = np.ascontiguousarray(np.stack([tri01, tri01], axis=1))  # [P, 2, P]
    ones2 = np.ones((P, 2, P), np.float16)
    zeros2 = np.zeros((P, 2, P), np.float16)

    in_maps = []
    for core in range(8):
        b, s = core // 2, core % 2
        xb = x[b]  # [T, C]
        own = np.concatenate([xb[(2 * j + s) * P : (2 * j + s + 1) * P] for j in range(CB)], 0)
        oth = np.concatenate([xb[(2 * j + 1 - s) * P : (2 * j + 2 - s) * P] for j in range(CB)], 0)
        xP = np.concatenate([own, oth], 0).T  # [C, T] permuted own-first
        x8i = _il8(xP, SX)
        pm = zeros2 if s == 0 else ones2
        in_maps.append(
            dict(
                x8=x8i,
                xo16=np.ascontiguousarray(own.T.astype(np.float16)),
                wq8=wq8, wk8=wk8, wv8=wv8, wp8=wp8,
                wqs=wqs_r, wks=wks_r, wvs=wvs_r,
                w1=w1r, w1s=w1sum, w2=w2h,
                msk=np.ascontiguousarray(np.stack([tri2, pm])),
            )
        )
    return in_maps


def _assemble(results):
    out = np.empty((B, T, C), np.float32)
    for core in range(8):
        b, s = core // 2, core % 2
        tokmajor = results[core]["outT"].T  # [TOWN, C]
        for j in range(CB):
            out[b, (2 * j + s) * P : (2 * j + s + 1) * P] = tokmajor[j * P : (j + 1) * P]
    return out


def kernel(**inputs):
    _install_wait_split()
    in_maps = _host_prep(inputs)
    nc = build_nc()
    res = run_bass_kernel_spmd(nc, in_maps, core_ids=list(range(8)))
    return _assemble(res.results)


if __name__ == "__main__":
    _install_wait_split()
    nc = build_nc()
    n = 0
    for bb in nc.m.functions[0].blocks:
        n += len(bb.instructions)
    print("built OK,", n, "instructions")


# revision 3
# speedup vs baseline: 1.0687x; 1.0687x over previous
"""Trainium2 Bass kernel v2 for one pre-LN transformer block (B=4, T=2048,
C=1024, H=16, HS=64, FFN=4096, causal).

Sharding: 8 cores = (batch b) x (parity s). Core (b, s) owns query blocks
{2j+s}. HOST PERMUTES tokens own-first: xP = [own 1024 | other 1024], so
the program is parity-independent (only input DATA differs per core).

Key structure vs v1:
  - LN1 folded into QKV: the GEMMs run directly on fp8-quantized raw x;
    mean correction is a K=1 matmul appended to each chain (stationary =
    colsum(W) row), 1/std is fused into the PSUM->SBUF copies (DVE row
    mult for Q/K, per-token [P,1] tensor_scalar for V). Stats are scaled
    by SX internally (x8 = x*SX) so all fold constants stay f16-normal.
  - QKV + proj in fp8e4m3 DoubleRow (two K-chunks interleaved per matmul);
    attention and FFN stay fp16 (fp8 FFN fails the error budget).
  - ACT only does exp (one instr covers both heads of a pair) and relu;
    copies are on DVE/Pool; causal masking is a DVE 0/1 multiply on the
    diagonal 128-chunk (no PE mask matmuls).
  - Permuted order makes segment starts exact-causal. Rowsum via ones
    column in V; reciprocal batched over all 16 heads; LN2 rsqrt applied
    on the FFN2 output (commutes through relu and the W2 contraction).
"""

import sys

for _p in ("/opt/trn_rl_repo", "/root/.axon_site/_ro/trn_rl_repo"):
    if _p not in sys.path:
        sys.path.append(_p)

import json
from contextlib import ExitStack

import numpy as np
import ml_dtypes

import concourse.bass as bass
import concourse.tile as tile
from concourse import mybir
from concourse.bass_utils import run_bass_kernel_spmd
from concourse.masks import make_identity

F32 = mybir.dt.float32
F16 = mybir.dt.float16
F8 = mybir.dt.float8e4
AF = mybir.ActivationFunctionType
OP = mybir.AluOpType
DR = mybir.MatmulPerfMode.DoubleRow

B, T, C, H, HS = 4, 2048, 1024, 16, 64
P = 128
CB = C // P            # 8 feature blocks
KT = CB // 2           # 4 interleaved K-pair chunks for fp8 DR
TB = T // P            # 16 token blocks
TOWN = T // 2          # own tokens per core
FF = 4 * C             # 4096
FB = FF // P           # 32
LN_EPS = 1e-5

# fp8 scales (powers of two; host asserts amax fits)
SX = 32.0              # x activations
SW = 1024.0            # all fp8 weights
SXO = 32.0             # attention output oT
ALPHA = 16.0           # extra kT/qT/v scale keeping f16 rows out of subnormals
CW = ALPHA / SW        # folds into r-scale rows (r comes out as r/SX)
EXP_SCALE = 0.125 / (ALPHA * ALPHA)
CP = 1.0 / (SXO * SW)  # proj dequant
SR = 16.0              # res1 fp8 scale (FFN1 inputs)
CF = 1.0 / (SR * SW)   # FFN1 dequant (inside relu)
FB8 = 32               # first FB8 of 32 FFN1 chunks run fp8 DR, rest fp16

_patched = False


def _install_wait_split():
    """Toolchain workarounds (same as v1): split multi-waits; drop the
    advisory birverifier pass."""
    global _patched
    if _patched:
        return
    _patched = True
    orig = bass.Bass.to_json_bytes

    def patched(self, *a, **kw):
        doc = json.loads(orig(self, *a, **kw))
        changed = False
        for f in doc.get("functions", []):
            for bb in f.get("basic_blocks", f.get("blocks", [])):
                out = []
                for inst in bb.get("instructions", []):
                    si = inst.get("sync_info")
                    waits = (si or {}).get("on_wait", [])
                    if len(waits) > 1:
                        changed = True
                        for k, w in enumerate(waits[:-1]):
                            out.append(
                                {
                                    "debug": inst.get("debug", 0),
                                    "engine": inst["engine"],
                                    "ins": [],
                                    "name": f"{inst['name']}_w{k}",
                                    "opcode": "EventSemaphore",
                                    "outs": [],
                                    "sync_info": {"on_update": [], "on_wait": [w]},
                                }
                            )
                        si["on_wait"] = waits[-1:]
                    out.append(inst)
                bb["instructions"] = out
        return json.dumps(doc).encode() if changed else orig(self, *a, **kw)

    bass.Bass.to_json_bytes = patched

    import concourse.bass_utils as bu

    orig_run = bu.run_command

    def patched_run(argv, **kw):
        argv = list(argv)
        for i, a in enumerate(argv):
            if isinstance(a, str) and a.startswith("birverifier,"):
                argv[i] = a[len("birverifier,"):]
        return orig_run(argv, **kw)

    bu.run_command = patched_run


def _seg512(lo, hi):
    segs = []
    while lo < hi:
        nxt = min(hi, (lo // 512 + 1) * 512)
        segs.append((lo, nxt))
        lo = nxt
    return segs


def build_nc(repeat=1, stop_after=None):
    nc = bass.Bass(target_bir_lowering=False)

    x8 = nc.dram_tensor("x8", [P, KT, 2, T], F8, kind="ExternalInput")
    xo16 = nc.dram_tensor("xo16", [C, TOWN], F16, kind="ExternalInput")
    wq8 = nc.dram_tensor("wq8", [P, KT, 2, C], F8, kind="ExternalInput")
    wk8 = nc.dram_tensor("wk8", [P, KT, 2, C], F8, kind="ExternalInput")
    wv8 = nc.dram_tensor("wv8", [P, KT, 2, C], F8, kind="ExternalInput")
    wp8 = nc.dram_tensor("wp8", [P, KT, 2, C], F8, kind="ExternalInput")
    wqs = nc.dram_tensor("wqs", [1, C], F16, kind="ExternalInput")
    wks = nc.dram_tensor("wks", [1, C], F16, kind="ExternalInput")
    wvs = nc.dram_tensor("wvs", [1, C], F16, kind="ExternalInput")
    w18 = nc.dram_tensor("w18", [P, KT, 2, FB8 * P], F8, kind="ExternalInput")
    w1 = (nc.dram_tensor("w1", [FB - FB8, C, P], F16, kind="ExternalInput")
          if FB8 < FB else None)
    w1s = nc.dram_tensor("w1s", [1, FF], F16, kind="ExternalInput")
    w2 = nc.dram_tensor("w2", [FF, C], F16, kind="ExternalInput")
    msk = nc.dram_tensor("msk", [2, P, 2, P], F16, kind="ExternalInput")
    outT = nc.dram_tensor("outT", [C, TOWN], F32, kind="ExternalOutput")

    with tile.TileContext(nc) as tc, ExitStack() as _rep, ExitStack() as top:
        if repeat > 1:
            _rep.enter_context(tc.For_i(0, repeat, 1))
        const = top.enter_context(tc.tile_pool(name="const", bufs=1, side="left"))
        ident16 = const.tile([P, P], F16, tag="ident")
        make_identity(nc, ident16)
        tri2 = const.tile([P, 2, P], F16, tag="tri2")
        pm2 = const.tile([P, 2, P], F16, tag="pm2")
        nc.sync.dma_start(out=tri2, in_=msk[0])
        nc.sync.dma_start(out=pm2, in_=msk[1])
        negon = const.tile([P, 1], F16, tag="negon")   # -1/C (stats on x8)
        poson = const.tile([P, 1], F16, tag="poson")   # sq tiles hold (x8/8)^2
        nc.vector.memset(negon, -1.0 / C)
        nc.vector.memset(poson, 64.0 / C)
        poson2 = const.tile([P, 1], F16, tag="poson2")
        nc.vector.memset(poson2, 1.0 / C)
        crow = const.tile([1, P], F16, tag="crow")     # ALPHA/SW for Q/K r-rows
        one_row = const.tile([1, P], F16, tag="onerow")
        nc.vector.memset(crow, CW)
        nc.vector.memset(one_row, 1.0)
        one_f32 = const.tile([1, 1], F32, tag="onef32")
        nc.vector.memset(one_f32, 1.0)
        eps1_sb = const.tile([1, 1], F32, tag="eps1")  # stats carry SX^2
        eps2_sb = const.tile([1, 1], F32, tag="eps2")
        nc.vector.memset(eps1_sb, LN_EPS * SX * SX)
        nc.vector.memset(eps2_sb, LN_EPS)

        def ln_rows(pool, nt, label):
            """Stats rows as [1, nt] single-partition tiles."""
            mneg = pool.tile([1, nt], F32, tag=f"m_{label}")
            work = pool.tile([1, nt], F32, tag=f"w_{label}")
            m16 = pool.tile([1, nt], F16, tag=f"m16_{label}")
            r16 = pool.tile([1, nt], F16, tag=f"r16_{label}")
            return mneg, work, m16, r16

        def ln_var_chain(mneg, work, m16, r16, eps_sb, sl=slice(None)):
            # in-place: m16=mneg; mneg<-mneg^2; work<-sqrt(work-mneg^2+eps);
            # work<-1/work (= rinv); r16=work
            nc.vector.tensor_copy(m16[0:1, sl], mneg[0:1, sl])
            nc.vector.tensor_tensor(out=mneg[0:1, sl], in0=mneg[0:1, sl],
                                    in1=mneg[0:1, sl], op=OP.mult)
            nc.vector.tensor_tensor(out=work[0:1, sl], in0=work[0:1, sl],
                                    in1=mneg[0:1, sl], op=OP.subtract)
            nc.scalar.activation(work[0:1, sl], work[0:1, sl], AF.Sqrt,
                                 bias=eps_sb[0:1, 0:1])
            nc.vector.reciprocal(out=work[0:1, sl], in_=work[0:1, sl])
            nc.vector.tensor_copy(r16[0:1, sl], work[0:1, sl])

        def replicate(row_ap, out_tile, out_sl, stat_row, parts, ps_pool, name):
            """out_tile[0:parts, out_sl] = stat_row_value * row_ap via K=1 PE."""
            rp = ps_pool.tile([parts, 512], F32, tag="repps", name=f"rp_{name}")
            nc.tensor.matmul(rp[0:parts, :], stat_row[0:1, 0:parts], row_ap,
                             start=True, stop=True)
            nc.scalar.activation(out_tile[0:parts, out_sl], rp[0:parts, :], AF.Copy)

        # Right-side pool stack, entered in REVERSE close order:
        # proj_grp & att_grp (close after proj) < qkv_grp (close after
        # attention) < x8_grp (close after QKV).
        proj_grp = ExitStack()
        wp_pool = proj_grp.enter_context(tc.tile_pool(name="wp8", bufs=1, side="right"))
        xo_pool = proj_grp.enter_context(tc.tile_pool(name="xo", bufs=CB, side="right"))
        att_grp = ExitStack()
        oT8_pool = att_grp.enter_context(tc.tile_pool(name="oT8", bufs=1, side="right"))
        oraw_pool = att_grp.enter_context(tc.tile_pool(name="oraw", bufs=2, side="right"))
        qkv_grp = ExitStack()
        qT_pool = qkv_grp.enter_context(tc.tile_pool(name="qT", bufs=CB, side="right"))
        kT_pool = qkv_grp.enter_context(tc.tile_pool(name="kT", bufs=CB, side="right"))
        v_pool = qkv_grp.enter_context(tc.tile_pool(name="v", bufs=TB, side="right"))
        x8_grp = ExitStack()
        x8sb_pool = x8_grp.enter_context(tc.tile_pool(name="x8sb", bufs=1, side="right"))
        x8sb = x8sb_pool.tile([P, KT, 2, T], F8, tag="x8")
        nc.sync.dma_start(out=x8sb, in_=x8[:, :, :, :])

        # raw stats on x8 = SX*x: mneg1 = -SX*m, work1 = SX^2*E[x^2],
        # r1 = 1/(SX*std); all fold constants account for this.
        rows1 = x8_grp.enter_context(tc.tile_pool(name="rows1", bufs=1, side="right"))
        mneg1, work1, m1_16, r1_16 = ln_rows(rows1, T, "1")
        r1c = rows1.tile([P, TB], F32, tag="r1c")

        reps = top.enter_context(tc.tile_pool(name="reps", bufs=1, side="left"))
        Rf = reps.tile([P, T], F16, tag="Rf")
        r1 = work1  # rinv lands in work1 after each seg's chain
        with tc.tile_pool(name="lnps", bufs=4, space="PSUM") as lnps, \
             tc.tile_pool(name="lnsq", bufs=2, side="right") as sqp, \
             tc.tile_pool(name="trps", bufs=1, space="PSUM") as trps, \
             tc.tile_pool(name="repps", bufs=2, space="PSUM") as repps:
            tcol = trps.tile([P, TB], F32, tag="tc")
            for n in range(4):
                sl = slice(n * 512, (n + 1) * 512)
                ps = lnps.tile([1, 512], F32, tag="st", name=f"l1a_{n}")
                ps2 = lnps.tile([1, 512], F32, tag="st", name=f"l1b_{n}")
                for t2 in range(KT):
                    for i in range(2):
                        nc.tensor.matmul(
                            ps[:], negon[:], x8sb[:, t2, i, sl],
                            start=(t2 == 0 and i == 0), stop=(t2 == KT - 1 and i == 1),
                        )
                        x8h = sqp.tile([P, 512], F8, tag="x8h", name=f"x8h_{n}_{t2}_{i}")
                        nc.scalar.activation(x8h[:], x8sb[:, t2, i, sl], AF.Copy,
                                             scale=0.125)
                        sq = sqp.tile([P, 512], F16, tag="sq", name=f"sq1_{n}_{t2}_{i}")
                        nc.vector.tensor_tensor(
                            out=sq[:], in0=x8h[:], in1=x8h[:], op=OP.mult,
                        )
                        nc.tensor.matmul(
                            ps2[:], poson[:], sq[:],
                            start=(t2 == 0 and i == 0), stop=(t2 == KT - 1 and i == 1),
                        )
                nc.scalar.activation(mneg1[0:1, sl], ps[:], AF.Copy)
                nc.scalar.activation(work1[0:1, sl], ps2[:], AF.Copy)
                # per-seg epilogue: var chain, Rf replicate, r-column transposes
                ln_var_chain(mneg1, work1, m1_16, r1_16, eps1_sb, sl)
                replicate(r1_16[0:1, sl], Rf, sl, crow, P, repps, f"Rf{n}")
                for tb in range(4 * n, 4 * n + 4):
                    nc.tensor.matmul(
                        tcol[:, tb : tb + 1], r1[0:1, tb * P : (tb + 1) * P],
                        one_f32[0:1, 0:1],
                        is_transpose=True, start=True, stop=True,
                    )
            nc.vector.tensor_scalar(
                out=r1c[:], in0=tcol[:], scalar1=CW, scalar2=None, op0=OP.mult
            )

        # ---- QKV GEMMs (fp8 DoubleRow chains + K=1 mean fold) ----
        wrow_pool = top.enter_context(tc.tile_pool(name="wrows", bufs=1, side="left"))
        wqs_sb = wrow_pool.tile([1, C], F16, tag="wqs")
        wks_sb = wrow_pool.tile([1, C], F16, tag="wks")
        wvs_sb = wrow_pool.tile([1, C], F16, tag="wvs")
        nc.sync.dma_start(out=wqs_sb, in_=wqs[:])
        nc.sync.dma_start(out=wks_sb, in_=wks[:])
        nc.sync.dma_start(out=wvs_sb, in_=wvs[:])
        # (w1s loads later, into rows2)

        qT_t = [qT_pool.tile([P, TOWN], F16, tag="qT", name=f"qT_{i}") for i in range(CB)]
        kT_t = [kT_pool.tile([P, T], F16, tag="kT", name=f"kT_{i}") for i in range(CB)]
        v_t = [v_pool.tile([P, 2, 8, HS + 1], F16, tag="v", name=f"v_{i}") for i in range(TB)]

        with ExitStack() as wgrp:
            w8_pool = wgrp.enter_context(tc.tile_pool(name="w8", bufs=2, side="right"))
            wk_sb = w8_pool.tile([P, KT, 2, C], F8, tag="w8", name="wk8sb")
            wq_sb = w8_pool.tile([P, KT, 2, C], F8, tag="w8", name="wq8sb")
            nc.sync.dma_start(out=wk_sb, in_=wk8[:, :, :, :])
            nc.sync.dma_start(out=wq_sb, in_=wq8[:, :, :, :])

            with tc.tile_pool(name="qkvps", bufs=6, space="PSUM") as qkvps:
                # K over full batch
                for p in range(CB):
                    po = slice(p * P, (p + 1) * P)
                    for n in range(4):
                        sl = slice(n * 512, (n + 1) * 512)
                        ps = qkvps.tile([P, 512], F32, tag="ps", name=f"kps_{p}_{n}")
                        for t2 in range(KT):
                            nc.tensor.matmul(
                                ps[:], wk_sb[:, t2, :, po], x8sb[:, t2, :, sl],
                                start=(t2 == 0), stop=False, perf_mode=DR,
                            )
                        nc.tensor.matmul(
                            ps[:], wks_sb[0:1, po], m1_16[0:1, sl],
                            start=False, stop=True,
                        )
                        nc.vector.tensor_tensor(
                            out=kT_t[p][:, sl], in0=ps[:], in1=Rf[:, sl], op=OP.mult
                        )
                # wv reuses wk's buffer once K is done
                wv_sb = w8_pool.tile([P, KT, 2, C], F8, tag="w8", name="wv8sb")
                nc.sync.dma_start(out=wv_sb, in_=wv8[:, :, :, :])
                # Q over own tokens (first 2 segs in permuted order)
                for p in range(CB):
                    po = slice(p * P, (p + 1) * P)
                    for n in range(2):
                        sl = slice(n * 512, (n + 1) * 512)
                        ps = qkvps.tile([P, 512], F32, tag="ps", name=f"qps_{p}_{n}")
                        for t2 in range(KT):
                            nc.tensor.matmul(
                                ps[:], wq_sb[:, t2, :, po], x8sb[:, t2, :, sl],
                                start=(t2 == 0), stop=False, perf_mode=DR,
                            )
                        nc.tensor.matmul(
                            ps[:], wqs_sb[0:1, po], m1_16[0:1, sl],
                            start=False, stop=True,
                        )
                        nc.vector.tensor_tensor(
                            out=qT_t[p][:, sl], in0=ps[:], in1=Rf[:, sl], op=OP.mult
                        )
                # V: stationary = x8 token chunk, moving = wv8 (8 heads/matmul)
                for tb in range(TB):
                    tsl = slice(tb * P, (tb + 1) * P)
                    mrow = m1_16[0:1, tb * P : (tb + 1) * P]
                    for hh in range(2):
                        hsl = slice(hh * 512, (hh + 1) * 512)
                        ps = qkvps.tile([P, 512], F32, tag="ps", name=f"vps_{tb}_{hh}")
                        for t2 in range(KT):
                            nc.tensor.matmul(
                                ps[:], x8sb[:, t2, :, tsl], wv_sb[:, t2, :, hsl],
                                start=(t2 == 0), stop=False, perf_mode=DR,
                            )
                        nc.tensor.matmul(
                            ps[:], mrow, wvs_sb[0:1, hsl],
                            start=False, stop=True,
                        )
                        nc.vector.tensor_scalar(
                            out=v_t[tb][:, hh, :, 0:HS],
                            in0=ps[:].rearrange("p (h d) -> p h d", h=8),
                            scalar1=r1c[:, tb : tb + 1], scalar2=None, op0=OP.mult,
                        )
                    nc.vector.memset(v_t[tb][:, :, :, HS : HS + 1], ALPHA)
        x8_grp.close()  # x8 SBUF freed after QKV

        if stop_after == "qkv":
            with tc.tile_pool(name="dbg", bufs=2, side="right") as dbg:
                for p in range(CB):
                    ob = dbg.tile([P, TOWN], F32, tag="o", name=f"dbgq_{p}")
                    nc.vector.tensor_copy(ob[:], kT_t[p][:, 0:TOWN])
                    nc.sync.dma_start(out=outT[p * P : (p + 1) * P, :], in_=ob[:])
            qkv_grp.close(); att_grp.close(); proj_grp.close()
            return nc

        # prefetch proj weights + residual x during attention
        wp_sb = wp_pool.tile([P, KT, 2, C], F8, tag="wp8")
        nc.sync.dma_start(out=wp_sb, in_=wp8[:, :, :, :])
        xo_t = [xo_pool.tile([P, TOWN], F16, tag="xo", name=f"xo_{c}") for c in range(CB)]
        for c in range(CB):
            nc.sync.dma_start(out=xo_t[c], in_=xo16[c * P : (c + 1) * P, :])

        # =========================== attention ============================
        oT8 = oT8_pool.tile([P, CB, TOWN], F8, tag="oT8")

        with tc.tile_pool(name="stps", bufs=2, space="PSUM") as stps, \
             tc.tile_pool(name="otps", bufs=2, space="PSUM") as otps, \
             tc.tile_pool(name="pt", bufs=6, side="right") as pt_pool, \
             tc.tile_pool(name="attnsb", bufs=3, side="right") as attnsb:
            def drain(p, ot):
                # inline PSUM reads: rowsum recip + raw copy; frees ot
                orawt = oraw_pool.tile([P, TOWN], F16, tag="oraw", name=f"oraw_{p}")
                recs = []
                for g in range(2):
                    h = 2 * p + g
                    off = g * 64
                    recf = attnsb.tile([1, TOWN], F32, tag="rec", name=f"rec_{h}")
                    nc.vector.reciprocal(out=recf[:], in_=ot[g][64:65, :])
                    nc.vector.tensor_copy(orawt[off : off + 64, :], ot[g][0:64, :])
                    recs.append(recf)
                return orawt, recs

            def make_tail(p, orawt, recs):
                def tail():
                    # deferred: rec16 -> rep (PE) -> oT8 = oraw * rep (DVE)
                    for g in range(2):
                        h = 2 * p + g
                        off = g * 64
                        rec16g = attnsb.tile([1, TOWN], F16, tag="rec16", name=f"rec16_{h}")
                        nc.vector.tensor_scalar(
                            out=rec16g[:], in0=recs[g][:], scalar1=SXO, scalar2=None,
                            op0=OP.mult,
                        )
                        rep = attnsb.tile([64, TOWN], F16, tag="rep", name=f"rep_{h}")
                        for n in range(2):
                            sl = slice(n * 512, (n + 1) * 512)
                            rp = stps.tile([P, 2, 512], F32, tag="st", name=f"rp_{h}_{n}")
                            nc.tensor.matmul(
                                rp[0:64, 0, :], one_row[0:1, 0:64], rec16g[0:1, sl],
                                start=True, stop=True,
                            )
                            nc.scalar.activation(rep[:, sl], rp[0:64, 0, :], AF.Copy)
                        nc.vector.tensor_tensor(
                            out=oT8[off : off + 64, p, :],
                            in0=orawt[off : off + 64, :], in1=rep[:], op=OP.mult,
                        )
                return tail

            pending = None
            for p in range(CB):
                ot = [
                    otps.tile([HS + 1, TOWN], F32, tag="ot", name=f"ot_{p}_{g}")
                    for g in range(2)
                ]
                for i in range(TB):
                    q0 = i * P if i < 8 else (i - 8) * P
                    first = True
                    for (lo, hi) in _seg512(q0, TOWN):
                        w = hi - lo
                        st = stps.tile([P, 2, 512], F32, tag="st", name=f"st_{p}_{i}_{lo}")
                        for g in range(2):
                            off = g * 64
                            nc.tensor.matmul(
                                st[:, g, 0:w],
                                kT_t[p][off : off + 64, i * P : (i + 1) * P],
                                qT_t[p][off : off + 64, lo:hi],
                                start=True, stop=True, skip_group_check=True,
                            )
                        pt = pt_pool.tile([P, 2, 512], F16, tag="pt", name=f"pt_{p}_{i}_{lo}")
                        nc.scalar.activation(
                            pt[:, :, 0:w], st[:, :, 0:w], AF.Exp, scale=EXP_SCALE
                        )
                        if first:
                            m = tri2 if i < 8 else pm2
                            nc.vector.tensor_tensor(
                                out=pt[:, :, 0:P], in0=pt[:, :, 0:P], in1=m[:],
                                op=OP.mult,
                            )
                            first = False
                        for g in range(2):
                            h = 2 * p + g
                            nc.tensor.matmul(
                                ot[g][:, lo:hi], v_t[i][:, h // 8, h % 8, :],
                                pt[:, g, 0:w],
                                start=(i == 0), stop=(i == TB - 1),
                                skip_group_check=True,
                            )
                    if i == 5 and pending is not None:
                        pending()
                        pending = None
                orawt, recs = drain(p, ot)
                pending = make_tail(p, orawt, recs)
            pending()

        qkv_grp.close()  # qT/kT/v freed

        if stop_after == "attn":
            with tc.tile_pool(name="dbg2", bufs=2, side="right") as dbg:
                for p in range(CB):
                    ob = dbg.tile([P, TOWN], F32, tag="o", name=f"dbga_{p}")
                    nc.vector.tensor_copy(ob[:], oT8[:, p, :])
                    nc.sync.dma_start(out=outT[p * P : (p + 1) * P, :], in_=ob[:])
            att_grp.close()
            proj_grp.close()
            return nc

        # ======================== proj + residual =========================
        res1_pool = top.enter_context(tc.tile_pool(name="res1", bufs=CB, side="left"))
        res18_pool = top.enter_context(tc.tile_pool(name="res18", bufs=1, side="left"))
        res1_t = [res1_pool.tile([P, TOWN], F16, tag="res1", name=f"res1_{i}") for i in range(CB)]
        res1_8 = res18_pool.tile([P, KT, 2, TOWN], F8, tag="res18")
        with tc.tile_pool(name="saps", bufs=6, space="PSUM") as saps, \
             tc.tile_pool(name="sa16p", bufs=4, side="right") as sa16p:
            for cp in range(CB):
                po = slice(cp * P, (cp + 1) * P)
                for n in range(2):
                    sl = slice(n * 512, (n + 1) * 512)
                    ps = saps.tile([P, 512], F32, tag="sa", name=f"saps_{cp}_{n}")
                    for t2 in range(KT):
                        nc.tensor.matmul(
                            ps[:], wp_sb[:, t2, :, po],
                            oT8[:, 2 * t2 : 2 * t2 + 2, sl],
                            start=(t2 == 0), stop=(t2 == KT - 1), perf_mode=DR,
                        )
                    sa16 = sa16p.tile([P, 512], F16, tag="s16", name=f"sa16_{cp}_{n}")
                    nc.scalar.activation(sa16[:], ps[:], AF.Copy, scale=CP)
                    nc.vector.tensor_tensor(
                        out=res1_t[cp][:, sl], in0=sa16[:], in1=xo_t[cp][:, sl],
                        op=OP.add,
                    )
                    nc.scalar.activation(
                        res1_8[:, cp // 2, cp % 2, sl], res1_t[cp][:, sl],
                        AF.Copy, scale=SR,
                    )
        att_grp.close()
        proj_grp.close()

        if stop_after == "proj":
            with tc.tile_pool(name="dbg3", bufs=2, side="right") as dbg:
                for c in range(CB):
                    ob = dbg.tile([P, TOWN], F32, tag="o", name=f"dbgp_{c}")
                    nc.vector.tensor_copy(ob[:], res1_t[c][:])
                    nc.sync.dma_start(out=outT[c * P : (c + 1) * P, :], in_=ob[:])
            return nc

        # ========================= LN2 + FFN1 =============================
        rows2 = top.enter_context(tc.tile_pool(name="rows2", bufs=1, side="left"))
        mneg2, work2, m2_16, r2_16 = ln_rows(rows2, TOWN, "2")
        with tc.tile_pool(name="ln2ps", bufs=4, space="PSUM") as lnps2, \
             tc.tile_pool(name="ln2sq", bufs=4, side="right") as sqp2:
            for n in range(2):
                sl = slice(n * 512, (n + 1) * 512)
                ps = lnps2.tile([1, 512], F32, tag="st", name=f"l2a_{n}")
                ps2 = lnps2.tile([1, 512], F32, tag="st", name=f"l2b_{n}")
                for c in range(CB):
                    nc.tensor.matmul(
                        ps[:], negon[:], res1_t[c][:, sl],
                        start=(c == 0), stop=(c == CB - 1),
                    )
                    sq = sqp2.tile([P, 512], F16, tag="sq", name=f"sq2_{n}_{c}")
                    nc.vector.tensor_tensor(
                        out=sq[:], in0=res1_t[c][:, sl], in1=res1_t[c][:, sl],
                        op=OP.mult,
                    )
                    nc.tensor.matmul(
                        ps2[:], poson2[:], sq[:],
                        start=(c == 0), stop=(c == CB - 1),
                    )
                nc.scalar.activation(mneg2[0:1, sl], ps[:], AF.Copy)
                nc.scalar.activation(work2[0:1, sl], ps2[:], AF.Copy)
        ln_var_chain(mneg2, work2, m2_16, r2_16, eps2_sb)

        R2 = reps.tile([P, TOWN], F16, tag="R2")
        with tc.tile_pool(name="r2ps", bufs=2, space="PSUM") as r2ps:
            for n in range(2):
                sl = slice(n * 512, (n + 1) * 512)
                replicate(r2_16[0:1, sl], R2, sl, one_row, P, r2ps, f"R2{n}")

        w1s_sb = rows2.tile([1, FF], F16, tag="w1s")
        nc.sync.dma_start(out=w1s_sb, in_=w1s[:])
        w1s8_sb = rows2.tile([1, FB8 * P], F16, tag="w1s8")
        nc.vector.tensor_scalar(
            out=w1s8_sb[:], in0=w1s_sb[0:1, 0 : FB8 * P], scalar1=SW * SR,
            scalar2=None, op0=OP.mult,
        )

        relu_pool = top.enter_context(tc.tile_pool(name="relu", bufs=FB, side="left"))
        relu_t = [relu_pool.tile([P, TOWN], F16, tag="relu", name=f"relu_{i}") for i in range(FB)]
        w2_pool = top.enter_context(tc.tile_pool(name="w2sb", bufs=4, side="right"))
        with tc.tile_pool(name="w18p", bufs=1, side="right") as w18_pool, \
             tc.tile_pool(name="w1fp", bufs=3, side="right") as w1f_pool, \
             tc.tile_pool(name="ups", bufs=6, space="PSUM") as ups:
            w18_sb = w18_pool.tile([P, KT, 2, FB8 * P], F8, tag="w18")
            nc.sync.dma_start(out=w18_sb, in_=w18[:, :, :, :])
            for fc in range(FB):
                if fc >= FB8:
                    w1f = w1f_pool.tile([P, CB, P], F16, tag="w1f", name=f"w1f_{fc}")
                    nc.sync.dma_start(
                        out=w1f, in_=w1[fc - FB8].rearrange("(cb p) f -> p cb f", p=P)
                    )
                for n in range(2):
                    sl = slice(n * 512, (n + 1) * 512)
                    ps = ups.tile([P, 512], F32, tag="u", name=f"ups_{fc}_{n}")
                    if fc < FB8:
                        for t2 in range(KT):
                            nc.tensor.matmul(
                                ps[:], w18_sb[:, t2, :, fc * P : (fc + 1) * P],
                                res1_8[:, t2, :, sl],
                                start=(t2 == 0), stop=False, perf_mode=DR,
                            )
                        nc.tensor.matmul(
                            ps[:], w1s8_sb[0:1, fc * P : (fc + 1) * P],
                            m2_16[0:1, sl],
                            start=False, stop=True,
                        )
                        nc.scalar.activation(relu_t[fc][:, sl], ps[:], AF.Relu,
                                             scale=CF)
                    else:
                        for c in range(CB):
                            nc.tensor.matmul(
                                ps[:], w1f[:, c, :], res1_t[c][:, sl],
                                start=(c == 0), stop=False,
                            )
                        nc.tensor.matmul(
                            ps[:], w1s_sb[0:1, fc * P : (fc + 1) * P],
                            m2_16[0:1, sl],
                            start=False, stop=True,
                        )
                        nc.scalar.activation(relu_t[fc][:, sl], ps[:], AF.Relu)

        if stop_after == "ffn1":
            with tc.tile_pool(name="dbg4", bufs=2, side="right") as dbg:
                for c in range(CB):
                    ob = dbg.tile([P, TOWN], F32, tag="o", name=f"dbgf_{c}")
                    nc.vector.tensor_copy(ob[:], relu_t[c][:])
                    nc.sync.dma_start(out=outT[c * P : (c + 1) * P, :], in_=ob[:])
            return nc

        # ============================ FFN2 ================================
        with tc.tile_pool(name="ffps", bufs=1, space="PSUM") as ffps, \
             tc.tile_pool(name="osb", bufs=4, side="right") as osb_pool:
            for th in range(2):
                sl = slice(th * 512, (th + 1) * 512)
                pss = [
                    ffps.tile([P, 512], F32, tag=f"ff{cp}", name=f"ffps_{th}_{cp}")
                    for cp in range(CB)
                ]
                for f4 in range(FB // 4):
                    w2t4 = w2_pool.tile([P, 4, C], F16, tag="w2", name=f"w2_{th}_{f4}")
                    nc.sync.dma_start(
                        out=w2t4,
                        in_=w2[f4 * 4 * P : (f4 + 1) * 4 * P, :].rearrange(
                            "(four p) c -> p four c", p=P
                        ),
                    )
                    for j in range(4):
                        fc = f4 * 4 + j
                        for cp in range(CB):
                            nc.tensor.matmul(
                                pss[cp][:], w2t4[:, j, cp * P : (cp + 1) * P],
                                relu_t[fc][:, sl],
                                start=(fc == 0), stop=(fc == FB - 1),
                            )
                for cp in range(CB):
                    ff16 = osb_pool.tile([P, 512], F16, tag="f16", name=f"ff16_{th}_{cp}")
                    nc.vector.tensor_tensor(
                        out=ff16[:], in0=pss[cp][:], in1=R2[:, sl], op=OP.mult
                    )
                    ob = osb_pool.tile([P, 512], F32, tag="ob", name=f"ob_{th}_{cp}")
                    nc.vector.tensor_tensor(
                        out=ob[:], in0=ff16[:], in1=res1_t[cp][:, sl], op=OP.add
                    )
                    nc.sync.dma_start(out=outT[cp * P : (cp + 1) * P, sl], in_=ob[:])

    return nc


# ---------------------------------------------------------------------------
# host side
# ---------------------------------------------------------------------------

E4NP = ml_dtypes.float8_e4m3  # trn2 fp8e4 is the IEEE-ish variant (max 240)


def _q8(a, scale):
    assert np.abs(a).max() * scale < 224.0, (float(np.abs(a).max()), scale)
    return (a * scale).astype(E4NP)


def _il8(m, scale):
    """[C, N] -> fp8 interleaved [P, KT, 2, N]."""
    Cd, N = m.shape
    assert Cd == C
    r = m.reshape(KT, 2, P, N).transpose(2, 0, 1, 3)
    return np.ascontiguousarray(_q8(r, scale))


def _host_prep(inputs):
    x = np.asarray(inputs["x"], np.float32)
    Wq = np.asarray(inputs["Wq"], np.float32)
    Wk = np.asarray(inputs["Wk"], np.float32)
    Wv = np.asarray(inputs["Wv"], np.float32)
    Wp = np.asarray(inputs["Wproj"], np.float32)
    W1 = np.asarray(inputs["W1"], np.float32)
    W2 = np.asarray(inputs["W2"], np.float32)

    wq2 = Wq.transpose(1, 0, 2).reshape(C, C)
    wk2 = Wk.transpose(1, 0, 2).reshape(C, C)
    wv2 = Wv.transpose(1, 0, 2).reshape(C, C)

    wq8 = _il8(wq2, SW)
    wk8 = _il8(wk2, SW)
    wv8 = _il8(wv2, SW)
    wp8 = _il8(Wp, SW)
    # mean-fold rows: chains produce SW*SX*(W^T x); m1_16 = -SX*m, so these
    # rows are colsum(W)*SW to land at SW*SX*(-m*colsum).
    wqs_r = np.ascontiguousarray((wq2.sum(0, dtype=np.float64) * SW).astype(np.float16)[None])
    wks_r = np.ascontiguousarray((wk2.sum(0, dtype=np.float64) * SW).astype(np.float16)[None])
    wvs_r = np.ascontiguousarray((wv2.sum(0, dtype=np.float64) * SW).astype(np.float16)[None])

    w18i = _il8(np.ascontiguousarray(W1[:, 0 : FB8 * P]), SW)
    w1sum = np.ascontiguousarray(W1.sum(0, dtype=np.float64).astype(np.float16)[None])
    w2h = np.ascontiguousarray(W2.astype(np.float16))

    tri01 = (np.arange(P)[:, None] <= np.arange(P)[None, :]).astype(np.float16)
    tri2 = np.ascontiguousarray(np.stack([tri01, tri01], axis=1))  # [P, 2, P]
    ones2 = np.ones((P, 2, P), np.float16)
    zeros2 = np.zeros((P, 2, P), np.float16)

    w1r = (np.ascontiguousarray(
        W1[:, FB8 * P :].reshape(C, FB - FB8, P).transpose(1, 0, 2).astype(np.float16))
        if FB8 < FB else None)
    in_maps = []
    for core in range(8):
        b, s = core // 2, core % 2
        xb = x[b]  # [T, C]
        own = np.concatenate([xb[(2 * j + s) * P : (2 * j + s + 1) * P] for j in range(CB)], 0)
        oth = np.concatenate([xb[(2 * j + 1 - s) * P : (2 * j + 2 - s) * P] for j in range(CB)], 0)
        xP = np.concatenate([own, oth], 0).T  # [C, T] permuted own-first
        x8i = _il8(xP, SX)
        pm = zeros2 if s == 0 else ones2
        in_maps.append(
            dict(
                x8=x8i,
                xo16=np.ascontiguousarray(own.T.astype(np.float16)),
                wq8=wq8, wk8=wk8, wv8=wv8, wp8=wp8,
                wqs=wqs_r, wks=wks_r, wvs=wvs_r,
                w18=w18i, w1s=w1sum, w2=w2h,
                msk=np.ascontiguousarray(np.stack([tri2, pm])),
                **({"w1": w1r} if w1r is not None else {}),
            )
        )
    return in_maps


def _assemble(results):
    out = np.empty((B, T, C), np.float32)
    for core in range(8):
        b, s = core // 2, core % 2
        tokmajor = results[core]["outT"].T  # [TOWN, C]
        for j in range(CB):
            out[b, (2 * j + s) * P : (2 * j + s + 1) * P] = tokmajor[j * P : (j + 1) * P]
    return out


def kernel(**inputs):
    _install_wait_split()
    in_maps = _host_prep(inputs)
    nc = build_nc()
    res = run_bass_kernel_spmd(nc, in_maps, core_ids=list(range(8)))
    return _assemble(res.results)


if __name__ == "__main__":
    _install_wait_split()
    nc = build_nc()
    n = 0
    for bb in nc.m.functions[0].blocks:
        n += len(bb.instructions)
    print("built OK,", n, "instructions")
